# revision 2
# baseline (speedup 1.0000x reference)
"""CRNN melody kernel: conv stack on 8 NeuronCores (data-parallel over batch),
segment-CSR max + BiLSTM tail on host.

Self-contained: hardcodes shapes from the problem spec.
  x            [8, 1, 80, 2048] f32
  tatum_frames [8, 257] i32 (CSR pointers, ptr[0]=0, ptr[-1]=2048)
  conv_w{i}/conv_b{i} for 6 layers; BiLSTM params w_ih_f/w_hh_f/b_f and _b.
Returns (out[..., :-1], out[..., -1]) like the reference.
"""
import numpy as np

B, T, F, N = 8, 2048, 80, 256
H = 130

_PMAP_CACHE = {}


def _conv_on_device(x, ws, bs):
    """Conv stack on the 8 trn2 cores, one batch element per core."""
    import jax
    import jax.numpy as jnp

    if "conv" not in _PMAP_CACHE:
        def conv_one(xb, wtup, btup):
            h = xb
            for w, b in zip(wtup, btup):
                h = jax.lax.conv_general_dilated(
                    h, w, (1, 1), "SAME",
                    dimension_numbers=("NCHW", "OIHW", "NCHW"),
                    precision=jax.lax.Precision.HIGHEST,
                )
                h = jax.nn.relu(h + b[None, :, None, None])
            return h

        _PMAP_CACHE["conv"] = jax.pmap(conv_one, in_axes=(0, None, None))

    pm = _PMAP_CACHE["conv"]
    # [B,1,F,T] -> [B,1,1,F,T]: one batch element per device
    xr = x.reshape(B, 1, 1, F, T)
    out = pm(xr, tuple(jnp.asarray(w) for w in ws), tuple(jnp.asarray(b) for b in bs))
    return np.asarray(out).reshape(B, 1, F, T)


def _conv_on_host(x, ws, bs):
    import jax
    import jax.numpy as jnp

    cpu = jax.devices("cpu")[0]
    with jax.default_device(cpu):
        h = jnp.asarray(x)
        for w, b in zip(ws, bs):
            h = jax.lax.conv_general_dilated(
                h, jnp.asarray(w), (1, 1), "SAME",
                dimension_numbers=("NCHW", "OIHW", "NCHW"),
            )
            h = jax.nn.relu(h + jnp.asarray(b)[None, :, None, None])
        return np.asarray(h)


def _sig(v):
    return 1.0 / (1.0 + np.exp(-v))


def _lstm(xs, w_ih, w_hh, b, reverse=False):
    # xs: [N, B, I] -> hs: [N, B, H]
    if reverse:
        xs = xs[::-1]
    xp = np.einsum("nbi,gi->nbg", xs, w_ih, dtype=np.float32) + b
    h = np.zeros((xs.shape[1], H), np.float32)
    c = np.zeros((xs.shape[1], H), np.float32)
    hs = np.empty((xs.shape[0], xs.shape[1], H), np.float32)
    w_hh_T = w_hh.T.astype(np.float32)
    for t in range(xs.shape[0]):
        g = xp[t] + h @ w_hh_T
        i, f, gg, o = np.split(g, 4, axis=-1)
        c = _sig(f) * c + _sig(i) * np.tanh(gg)
        h = _sig(o) * np.tanh(c)
        hs[t] = h
    return hs[::-1] if reverse else hs


def kernel(x, tatum_frames, conv_w0, conv_b0, conv_w1, conv_b1, conv_w2, conv_b2,
           conv_w3, conv_b3, conv_w4, conv_b4, conv_w5, conv_b5,
           w_ih_f, w_hh_f, b_f, w_ih_b, w_hh_b, b_b):
    x = np.asarray(x, np.float32)
    ptr = np.asarray(tatum_frames)
    ws = [np.asarray(w, np.float32) for w in
          (conv_w0, conv_w1, conv_w2, conv_w3, conv_w4, conv_w5)]
    bs = [np.asarray(b, np.float32) for b in
          (conv_b0, conv_b1, conv_b2, conv_b3, conv_b4, conv_b5)]

    if _PMAP_CACHE.get("dev_broken"):
        feats = _conv_on_host(x, ws, bs)
    else:
        try:
            feats = _conv_on_device(x, ws, bs)
        except Exception:
            _PMAP_CACHE["dev_broken"] = True
            feats = _conv_on_host(x, ws, bs)

    feats = feats.reshape(B, F, T)

    # per-batch segment_csr max: pooled[b, n, f] = max over t in [ptr[n], ptr[n+1])
    pooled = np.empty((B, N, F), np.float32)
    for b in range(B):
        starts = np.asarray(ptr[b][:-1], np.int64)
        pooled[b] = np.maximum.reduceat(feats[b].T, starts, axis=0)

    seq = pooled.transpose(1, 0, 2)  # [N, B, F]
    hf = _lstm(seq, np.asarray(w_ih_f, np.float32), np.asarray(w_hh_f, np.float32),
               np.asarray(b_f, np.float32), reverse=False)
    hb = _lstm(seq, np.asarray(w_ih_b, np.float32), np.asarray(w_hh_b, np.float32),
               np.asarray(b_b, np.float32), reverse=True)
    out = (hf + hb).transpose(1, 0, 2).astype(np.float32)  # [B, N, H]
    return out[..., :-1], out[..., -1]


# revision 3
# speedup vs baseline: 22.7365x; 22.7365x over previous
"""CRNN melody kernel: conv stack on 8 NeuronCores (data-parallel over batch),
segment-CSR max + BiLSTM tail on host.

Self-contained: hardcodes shapes from the problem spec.
  x            [8, 1, 80, 2048] f32
  tatum_frames [8, 257] i32 (CSR pointers, ptr[0]=0, ptr[-1]=2048)
  conv_w{i}/conv_b{i} for 6 layers; BiLSTM params w_ih_f/w_hh_f/b_f and _b.
Returns (out[..., :-1], out[..., -1]) like the reference.
"""
import numpy as np

B, T, F, N = 8, 2048, 80, 256
H = 130

_PMAP_CACHE = {}


def _conv_on_device(x, ws, bs):
    """Conv stack on the 8 trn2 cores, one batch element per core."""
    import jax
    import jax.numpy as jnp

    if "conv" not in _PMAP_CACHE:
        def conv_one(xb, wtup, btup):
            # xb [1, C, F, T]; conv as sum of shifted channel-matmuls
            h = xb[0]  # [C, F, T]
            for w, b in zip(wtup, btup):
                k = w.shape[-1]
                p = k // 2
                hp = jnp.pad(h, ((0, 0), (p, p), (p, p)))
                acc = jnp.zeros((w.shape[0], F, T), jnp.float32)
                for i in range(k):
                    for j in range(k):
                        acc = acc + jnp.einsum(
                            "oc,cft->oft", w[:, :, i, j],
                            jax.lax.slice(hp, (0, i, j),
                                          (hp.shape[0], i + F, j + T)),
                            precision=jax.lax.Precision.HIGHEST,
                        )
                h = jax.nn.relu(acc + b[:, None, None])
            return h[None]

        _PMAP_CACHE["conv"] = jax.pmap(conv_one, in_axes=(0, None, None))

    pm = _PMAP_CACHE["conv"]
    # [B,1,F,T] -> [B,1,1,F,T]: one batch element per device
    xr = x.reshape(B, 1, 1, F, T)
    out = pm(xr, tuple(jnp.asarray(w) for w in ws), tuple(jnp.asarray(b) for b in bs))
    return np.asarray(out).reshape(B, 1, F, T)


def _conv_on_host(x, ws, bs):
    import jax
    import jax.numpy as jnp

    cpu = jax.devices("cpu")[0]
    with jax.default_device(cpu):
        h = jnp.asarray(x)
        for w, b in zip(ws, bs):
            h = jax.lax.conv_general_dilated(
                h, jnp.asarray(w), (1, 1), "SAME",
                dimension_numbers=("NCHW", "OIHW", "NCHW"),
            )
            h = jax.nn.relu(h + jnp.asarray(b)[None, :, None, None])
        return np.asarray(h)


def _sig(v):
    return 1.0 / (1.0 + np.exp(-v))


def _lstm(xs, w_ih, w_hh, b, reverse=False):
    # xs: [N, B, I] -> hs: [N, B, H]
    if reverse:
        xs = xs[::-1]
    xp = np.einsum("nbi,gi->nbg", xs, w_ih, dtype=np.float32) + b
    h = np.zeros((xs.shape[1], H), np.float32)
    c = np.zeros((xs.shape[1], H), np.float32)
    hs = np.empty((xs.shape[0], xs.shape[1], H), np.float32)
    w_hh_T = w_hh.T.astype(np.float32)
    for t in range(xs.shape[0]):
        g = xp[t] + h @ w_hh_T
        i, f, gg, o = np.split(g, 4, axis=-1)
        c = _sig(f) * c + _sig(i) * np.tanh(gg)
        h = _sig(o) * np.tanh(c)
        hs[t] = h
    return hs[::-1] if reverse else hs


def kernel(x, tatum_frames, conv_w0, conv_b0, conv_w1, conv_b1, conv_w2, conv_b2,
           conv_w3, conv_b3, conv_w4, conv_b4, conv_w5, conv_b5,
           w_ih_f, w_hh_f, b_f, w_ih_b, w_hh_b, b_b):
    x = np.asarray(x, np.float32)
    ptr = np.asarray(tatum_frames)
    ws = [np.asarray(w, np.float32) for w in
          (conv_w0, conv_w1, conv_w2, conv_w3, conv_w4, conv_w5)]
    bs = [np.asarray(b, np.float32) for b in
          (conv_b0, conv_b1, conv_b2, conv_b3, conv_b4, conv_b5)]

    if _PMAP_CACHE.get("dev_broken"):
        feats = _conv_on_host(x, ws, bs)
    else:
        try:
            feats = _conv_on_device(x, ws, bs)
        except Exception:
            _PMAP_CACHE["dev_broken"] = True
            feats = _conv_on_host(x, ws, bs)

    feats = feats.reshape(B, F, T)

    # per-batch segment_csr max: pooled[b, n, f] = max over t in [ptr[n], ptr[n+1])
    pooled = np.empty((B, N, F), np.float32)
    for b in range(B):
        starts = np.asarray(ptr[b][:-1], np.int64)
        pooled[b] = np.maximum.reduceat(feats[b].T, starts, axis=0)

    seq = pooled.transpose(1, 0, 2)  # [N, B, F]
    hf = _lstm(seq, np.asarray(w_ih_f, np.float32), np.asarray(w_hh_f, np.float32),
               np.asarray(b_f, np.float32), reverse=False)
    hb = _lstm(seq, np.asarray(w_ih_b, np.float32), np.asarray(w_hh_b, np.float32),
               np.asarray(b_b, np.float32), reverse=True)
    out = (hf + hb).transpose(1, 0, 2).astype(np.float32)  # [B, N, H]
    return out[..., :-1], out[..., -1]
